# revision 51
# baseline (speedup 1.0000x reference)
"""Trainium2 Bass kernel for nn_Attention_45835890982922.

Dense multi-head attention block:
    qkv = x @ w_qkv ; q,k layernormed per head (eps=1e-5), q scaled by D^-0.5
    out = softmax(q k^T) v ; y = concat_heads(out) @ w_proj + b_proj

Sharding over 8 NeuronCores: hybrid batch x tensor-parallel.
Core c handles batch b = c//2 and heads [6*(c%2), 6*(c%2)+6).
Each core computes a partial y^T (its 6 heads through the matching
w_proj rows); the host sums the two partials per batch and adds b_proj.

On-chip layout is feature-major (transposed): x^T, q^T, k^T [D, tokens],
so every matmul contraction lives on the partition axis with no per-head
transposes.  Softmax runs without max-subtraction (|S| <= ~8 after LN),
with the normalization sum obtained from an extra all-ones column
appended to v; the division is folded into the PSUM->SBUF epilogue of
the attention-output matmul.

Fast path (beta=0, per-channel gamma constant): the k-side LayerNorm
mean subtraction is algebraically eliminated.  Because LN centers q,
sum_d(qhat_d) = 0, so the k-mean term of S cancels:
    S = ((q - mu_q) * w)[query] . (k_raw * u)[key]
with w = scale*gamma_q*rstd_q folded into the q apply and u = gamma_k *
rstd_k multiplied into the raw k columns (cheaper than the exp
activation's per-partition scale operand, which costs ~400ns/op on HW).
rstd is computed as exp(-0.5*ln(var+eps) + ln(const)) so every ACT op
(ln/exp/square) stays in one activation table set - no table reloads.
q and k stats share [6, 2, N] tiles (packed along the free dim), so one
smalls chain serves both sides; the per-partition exp bias supplies the
differing q/k constants.  The v matmuls + LN applies bridge the gap
between the stats chain and the ACT-bound attention phase so the PE
stays busy while the smalls chain runs on ACT/DVE.

dtypes: float32r (TensorE reduced fp32, ~1.5e-4) for qkv/S/stats/proj
matmuls, bf16 for exp(S) probabilities and v, fp32 accumulation in PSUM.
"""

import math
from contextlib import ExitStack

import numpy as np

import concourse.bacc as bacc
import concourse.tile as tile
import concourse.mybir as mybir
from concourse.bass_utils import run_bass_kernel_spmd

F32 = mybir.dt.float32
F32R = mybir.dt.float32r
BF16 = mybir.dt.bfloat16
OP = mybir.AluOpType
AF = mybir.ActivationFunctionType

B, N, C, H, D = 4, 2048, 768, 12, 64
HL = H // 2              # 6 heads per core
CL = HL * D              # 384 local feature rows
P = 128
NKT = N // P             # 16 key tiles
QC = 1024                # query chunk for attention
NQC = N // QC
CT = C // P              # 6 contraction tiles over C
FT_QK = 2 * CL // P      # 6 feature tiles for q|k
KT3 = CL // P            # 3 contraction tiles over CL
LN_EPS = 1e-5
SCALE = float(D) ** -0.5
KOFF = 32                # partition offset of k rows in the shared smalls

# ablation knobs (timing experiments only; wrong output when not default)
ABL_HEADS = HL
ABL_QKV = True
ABL_ATTN = True
ABL_PROJ = True
ABL_EXPFN = True
ABL_LN = True
QKV_PIPE = False         # interleaving qkv with transposes is slower on HW
                         # (6-way PSUM group interleave serializes the PE)
ABL_EXPW = QC            # exp free-width (timing probe; wrong output if < QC)
PO_W = QC                # PV psum window width (512 frees banks for PSS_BUFS=3)
EXP_IMMB = True          # emit softmax exp with immediate bias (no const AP)
MMW = 512                # matmul moving-dim width; HW caps at one PSUM bank
                         # (walrus rejects 1024-wide matmuls: invalid ISA)
PSS_BUFS = 2
PSO_BUFS = 2
HPARTS = 4
ABL_EPI = True


class _FtView:
    """view[p_slice, ft, col_slice] -> per-ft tile AP."""
    def __init__(self, tiles):
        self.tiles = tiles

    def __getitem__(self, idx):
        p, ft, col = idx
        return self.tiles[ft][p, col]


def _act_imm(nc, out, in_, func):
    """InstActivation with immediate bias/scale (bypasses bass's forced
    bias->const-AP conversion; per-partition operand reads cost ~400ns/op
    on the HW ACT engine)."""
    eng = nc.scalar
    ins = [
        eng.lower_ap(in_),
        mybir.ImmediateValue(dtype=mybir.dt.float32, value=0.0),
        mybir.ImmediateValue(dtype=mybir.dt.float32, value=1.0),
        mybir.ImmediateValue(dtype=mybir.dt.float32, value=0.0),
    ]
    return eng.add_instruction(
        mybir.InstActivation(
            name=nc.get_next_instruction_name(),
            func=func,
            ins=ins,
            outs=[eng.lower_ap(out)],
        )
    )


def _build(mode, repeat: int = 1):
    """mode: True -> fast path with gamma=1, beta=0;
    dict(cq=..., ck=...) -> fast path with constant gammas folded;
    False -> general path (per-channel gamma, beta supported)."""
    if mode is True:
        mode = {"cq": math.log(SCALE), "ck": 0.0}
    fast = mode is not False

    nc = bacc.Bacc("TRN2", target_bir_lowering=False, debug=False, num_devices=8)

    x_d = nc.dram_tensor("x", [N, C], F32R, kind="ExternalInput").ap()
    wqk_d = nc.dram_tensor("wqk", [C, 2 * CL], F32R, kind="ExternalInput").ap()
    wv_d = nc.dram_tensor("wv", [C, CL], F32R, kind="ExternalInput").ap()
    wp_d = nc.dram_tensor("wp", [CL, C], F32R, kind="ExternalInput").ap()
    ident_d = nc.dram_tensor("ident", [P, P], F32R, kind="ExternalInput").ap()
    bd6_d = nc.dram_tensor("bd6", [CL, HL], F32R, kind="ExternalInput").ap()
    bc6_d = nc.dram_tensor("bc6", [HL, CL], F32R, kind="ExternalInput").ap()
    gb_d = nc.dram_tensor("gb", [CL, 4], F32, kind="ExternalInput").ap()
    y_d = nc.dram_tensor("y", [C, N], F32, kind="ExternalOutput").ap()

    with tile.TileContext(nc) as tc, ExitStack() as top:
        top.enter_context(
            nc.allow_low_precision(reason="f32r/bf16 staging is intentional")
        )
        const = top.enter_context(tc.tile_pool(name="const", bufs=1))

        ident = const.tile([P, P], F32R)
        nc.sync.dma_start(ident[:], ident_d)
        bd6 = const.tile([P, KT3, HL], F32R)
        nc.sync.dma_start(bd6[:], bd6_d.rearrange("(t p) h -> p t h", p=P))
        bc6 = const.tile([HL, CL], F32R)
        nc.sync.dma_start(bc6[:], bc6_d)
        gb = const.tile([P, KT3, 4], F32)
        nc.sync.dma_start(gb[:], gb_d.rearrange("(t p) c -> p t c", p=P))
        cb = None
        if fast:
            cb = const.tile([HL, 2], F32)
            nc.gpsimd.memset(cb[:, 0:1], mode["cq"])
            nc.gpsimd.memset(cb[:, 1:2], mode["ck"])

        for rep in range(repeat):
            emit = _emit_iteration_fast if fast else _emit_iteration_general
            emit(nc, tc, rep, cb, x_d, wqk_d, wv_d, wp_d, y_d,
                 ident, bd6, bc6, gb)

    nc.compile()
    return nc


def _emit_iteration_fast(nc, tc, rep, cb,
                         x_d, wqk_d, wv_d, wp_d, y_d, ident, bd6, bc6, gb):
    with ExitStack() as top:
        vp = top.enter_context(tc.tile_pool(name=f"vpool{rep}", bufs=1))
        # v token-major bf16 with per-head all-ones column: [p, ttile, h*65+e]
        v_sb = vp.tile([P, NKT, HL * 65], BF16)
        v_view = v_sb[:].rearrange("p t (h e) -> p t h e", h=HL)
        nc.gpsimd.memset(v_view[:, :, :, 64:65], 1.0)

        # q^T | k^T feature-major accumulator: [p, ft, tokens]; ft 0-2 q, 3-5 k.
        # LayerNorm is applied in-place on q fts only; k fts stay raw.
        qkp = top.enter_context(tc.tile_pool(name=f"qkraw{rep}", bufs=1))
        qk_fts = [
            qkp.tile([P, N], F32R, name=f"qk_ft{ft}_{rep}") for ft in range(FT_QK)
        ]
        qk_raw = _FtView(qk_fts)
        hat = qk_raw

        # ================ phase A1: x^T + qkv, DMA-pipelined ================
        # The first nk pass interleaves the per-ct transposes with that ct's
        # qkv partial matmuls so the PE tracks the x DMA instead of stalling.
        pAC = top.enter_context(ExitStack())
        pA = pAC.enter_context(tc.tile_pool(name=f"phA{rep}", bufs=1))
        wqk_r = pA.tile([P, CT, 2 * CL], F32R)
        nc.sync.dma_start(wqk_r[:], wqk_d.rearrange("(t p) f -> p t f", p=P))
        wv_r = pA.tile([P, CT, CL], F32R)
        nc.sync.dma_start(wv_r[:], wv_d.rearrange("(t p) f -> p t f", p=P))
        x_t = pA.tile([P, CT, N], F32R)               # [c%128, ctile, token]

        sAC = top.enter_context(ExitStack())
        smp = sAC.enter_context(tc.tile_pool(name=f"smalls{rep}", bufs=1))
        with ExitStack() as sA:
            pAx = sA.enter_context(tc.tile_pool(name=f"phAx{rep}", bufs=2))
            psT = sA.enter_context(tc.tile_pool(name=f"psT{rep}", bufs=2, space="PSUM"))
            psQ = sA.enter_context(tc.tile_pool(name=f"psQ{rep}", bufs=1, space="PSUM"))
            if QKV_PIPE:
                for nk in range(N // 512):
                    psq = [
                        psQ.tile([P, 512], F32, tag=f"q{ft}", name=f"psq{ft}_{rep}_{nk}")
                        for ft in range(FT_QK)
                    ]
                    for ct in range(CT):
                        if nk == 0:
                            xs = pAx.tile([P, NKT, P], F32R, tag="xslice")
                            nc.sync.dma_start(
                                xs[:],
                                x_d.rearrange("(t p) c -> p t c", p=P)[
                                    :, :, ct * P:(ct + 1) * P
                                ],
                            )
                            for tg in range(NKT // 4):
                                pst = psT.tile([P, 4 * P], F32R, tag="ps_tr")
                                for i in range(4):
                                    nc.tensor.transpose(
                                        pst[:, i * P:(i + 1) * P],
                                        xs[:, tg * 4 + i, :], ident[:],
                                    )
                                nc.vector.tensor_copy(
                                    x_t[:, ct, tg * 512:(tg + 1) * 512], pst[:]
                                )
                        for ft in range(FT_QK if ABL_QKV else 0):
                            nc.tensor.matmul(
                                psq[ft][:],
                                wqk_r[:, ct, ft * P:(ft + 1) * P],
                                x_t[:, ct, nk * 512:(nk + 1) * 512],
                                start=(ct == 0),
                                stop=(ct == CT - 1),
                            )
                    for ft in range(FT_QK if ABL_QKV else 0):
                        nc.vector.tensor_copy(
                            qk_raw[:, ft, nk * 512:(nk + 1) * 512], psq[ft][:]
                        )
            else:
                for ct in range(CT):
                    xs = pAx.tile([P, NKT, P], F32R, tag="xslice")
                    nc.sync.dma_start(
                        xs[:],
                        x_d.rearrange("(t p) c -> p t c", p=P)[
                            :, :, ct * P:(ct + 1) * P
                        ],
                    )
                    for tg in range(NKT // 4):
                        pst = psT.tile([P, 4 * P], F32R, tag="ps_tr")
                        for i in range(4):
                            nc.tensor.transpose(
                                pst[:, i * P:(i + 1) * P],
                                xs[:, tg * 4 + i, :], ident[:],
                            )
                        nc.vector.tensor_copy(
                            x_t[:, ct, tg * 512:(tg + 1) * 512], pst[:]
                        )
                for ft in range(FT_QK if ABL_QKV else 0):
                    for nk in range(N // 512):
                        ps = psT.tile([P, 512], F32, tag="ps_qkv", name=f"ps_{rep}_{ft}_{nk}")
                        for kt in range(CT):
                            nc.tensor.matmul(
                                ps[:],
                                wqk_r[:, kt, ft * P:(ft + 1) * P],
                                x_t[:, kt, nk * 512:(nk + 1) * 512],
                                start=(kt == 0),
                                stop=(kt == CT - 1),
                            )
                        nc.vector.tensor_copy(
                            qk_raw[:, ft, nk * 512:(nk + 1) * 512], ps[:]
                        )

        # ================ phase B: LN stats (q side 0, k side 1) =============
        # q and k stats share [6, 2, N] tiles (packed along the free dim) so
        # one smalls chain serves both sides; only the final Exp splits, to
        # supply the differing q/k bias constants.
        sm_mu = smp.tile([HL, 2, N], F32R, name=f"sm_mu{rep}")
        sm_rst = smp.tile([HL, 2, N], F32R, name=f"sm_rst{rep}")
        with ExitStack() as sB:
            pB = sB.enter_context(tc.tile_pool(name=f"phB{rep}", bufs=2))
            psB = sB.enter_context(tc.tile_pool(name=f"psB{rep}", bufs=2, space="PSUM"))
            for s in range(2 if ABL_LN else 0):
                for nh in range(N // 1024):
                    psm = psB.tile([HL, 1024], F32, tag="ps_stat")
                    for half in range(2):
                        nk = nh * 2 + half
                        for kt in range(KT3):
                            nc.tensor.matmul(
                                psm[:, half * 512:(half + 1) * 512],
                                bd6[:, kt, :],
                                qk_raw[:, 3 * s + kt, nk * 512:(nk + 1) * 512],
                                start=(kt == 0),
                                stop=(kt == KT3 - 1),
                            )
                    nc.vector.tensor_scalar_mul(
                        sm_mu[:, s, nh * 1024:(nh + 1) * 1024], psm[:], 1.0 / D
                    )
            for s in range(2 if ABL_LN else 0):
                for nh in range(N // 1024):
                    psm = psB.tile([HL, 1024], F32, tag="ps_stat")
                    for half in range(2):
                        nk = nh * 2 + half
                        for kt in range(KT3):
                            sq = pB.tile([P, 512], F32R, tag="sq")
                            _act_imm(
                                nc, sq[:],
                                qk_raw[:, 3 * s + kt, nk * 512:(nk + 1) * 512],
                                AF.Square,
                            )
                            nc.tensor.matmul(
                                psm[:, half * 512:(half + 1) * 512],
                                bd6[:, kt, :],
                                sq[:],
                                start=(kt == 0),
                                stop=(kt == KT3 - 1),
                            )
                    nc.vector.tensor_scalar_mul(
                        sm_rst[:, s, nh * 1024:(nh + 1) * 1024], psm[:], 1.0 / D
                    )
            if ABL_LN:
                # rstd*const = exp(-0.5*ln(var+eps) + cb); one chain for both
                # q and k; ln/exp share the softmax exp's ACT table.
                tmp = smp.tile([HL, 2, N], F32, name=f"tmp{rep}")
                nc.vector.tensor_tensor(tmp[:], sm_mu[:], sm_mu[:], OP.mult)
                nc.vector.scalar_tensor_tensor(
                    tmp[:], sm_rst[:], LN_EPS, tmp[:],
                    op0=OP.add, op1=OP.subtract,
                )
                _act_imm(nc, tmp[:], tmp[:], AF.Ln)
                for s in range(2):
                    nc.scalar.activation(
                        sm_rst[:, s, :], tmp[:, s, :], AF.Exp, scale=-0.5,
                        bias=cb[:, s:s + 1],
                    )

        # ========= bridge: v matmuls + q apply + k rstd scale ===========
        # v fills the PE while the smalls chain runs on ACT/DVE; the ft0
        # apply and the head-0/1 k scale land between the two v halves so
        # attention can start the moment they retire.  rstd_k is multiplied
        # into the raw k columns here instead of riding the exp scale AP -
        # the per-partition-scale ACT mode costs ~400ns per exp on HW.
        with ExitStack() as sV:
            pC = sV.enter_context(tc.tile_pool(name=f"phC{rep}", bufs=2))
            psV = sV.enter_context(tc.tile_pool(name=f"psV{rep}", bufs=2, space="PSUM"))
            psC = sV.enter_context(tc.tile_pool(name=f"psC{rep}", bufs=1, space="PSUM"))

            def emit_v(tts):
                # PSUM->SBUF copy on ACT (idle here) so the DVE apply/kscale
                # chain isn't delayed - attention starts on those.
                for tt in tts:
                    psv = psV.tile([P, CL], F32, tag="ps_v")
                    for kt in range(CT):
                        nc.tensor.matmul(
                            psv[:],
                            x_t[:, kt, tt * P:(tt + 1) * P],
                            wv_r[:, kt, :],
                            start=(kt == 0),
                            stop=(kt == CT - 1),
                        )
                    _act_imm(
                        nc,
                        v_view[:, tt, :, 0:64],
                        psv[:].rearrange("p (h d) -> p h d", h=HL),
                        AF.Copy,
                    )

            def emit_apply(ft):
                # qhat = (q - mu_bcast) * w_bcast, in place
                for nh in range(N // 1024):
                    sl = slice(nh * 1024, (nh + 1) * 1024)
                    bmu = psC.tile([P, 1024], F32, tag="bmu")
                    brs = psC.tile([P, 1024], F32, tag="brs")
                    for half in range(2):
                        hs = slice(half * 512, (half + 1) * 512)
                        gs = slice(nh * 1024 + half * 512,
                                   nh * 1024 + (half + 1) * 512)
                        nc.tensor.matmul(
                            bmu[:, hs], bc6[:, ft * P:(ft + 1) * P],
                            sm_mu[:, 0, gs], start=True, stop=True,
                        )
                        nc.tensor.matmul(
                            brs[:, hs], bc6[:, ft * P:(ft + 1) * P],
                            sm_rst[:, 0, gs], start=True, stop=True,
                        )
                    tdiff = pC.tile([P, 1024], F32, tag="tdiff")
                    nc.vector.tensor_tensor(
                        tdiff[:], qk_raw[:, ft, sl], bmu[:], OP.subtract
                    )
                    nc.vector.tensor_tensor(
                        hat[:, ft, sl], tdiff[:], brs[:], OP.mult
                    )

            def emit_kscale(blk):
                # ktilde = k_raw * rstd_k_bcast, in place (ft = 3 + blk)
                for nh in range(N // 1024):
                    sl = slice(nh * 1024, (nh + 1) * 1024)
                    kbr = psC.tile([P, 1024], F32, tag="kbrs")
                    for half in range(2):
                        hs = slice(half * 512, (half + 1) * 512)
                        gs = slice(nh * 1024 + half * 512,
                                   nh * 1024 + (half + 1) * 512)
                        nc.tensor.matmul(
                            kbr[:, hs], bc6[:, blk * P:(blk + 1) * P],
                            sm_rst[:, 1, gs], start=True, stop=True,
                        )
                    nc.vector.tensor_tensor(
                        hat[:, 3 + blk, sl], qk_raw[:, 3 + blk, sl], kbr[:],
                        OP.mult,
                    )

            emit_v(range(0, NKT // 2))
            if ABL_LN:
                emit_apply(0)
                emit_kscale(0)
            emit_v(range(NKT // 2, NKT))
            if ABL_LN:
                emit_apply(1)
                emit_kscale(1)
                emit_apply(2)
                emit_kscale(2)

        sAC.close()  # free LN smalls before attention
        pAC.close()  # free x_t / weight staging before attention

        # ================ phase D: attention ================
        outp = top.enter_context(tc.tile_pool(name=f"outT{rep}", bufs=1))
        out_fts = [
            outp.tile([P, N], F32R, name=f"out_ft{t}_{rep}") for t in range(KT3)
        ]
        out_t = _FtView(out_fts)                      # out^T feature-major
        with ExitStack() as sD:
            expp = sD.enter_context(tc.tile_pool(name=f"expp{rep}", bufs=2 * HPARTS))
            pD = sD.enter_context(tc.tile_pool(name=f"phD{rep}", bufs=2))
            psS = sD.enter_context(tc.tile_pool(name=f"psS{rep}", bufs=PSS_BUFS, space="PSUM"))
            psO = sD.enter_context(tc.tile_pool(name=f"psO{rep}", bufs=PSO_BUFS, space="PSUM"))
            HK = NKT // HPARTS

            def emit_s_exp(h, qc):
                ht = h // 2
                hr = 64 * (h % 2)
                exp_halves = []
                for half in range(HPARTS):
                    exp_h = expp.tile(
                        [P, HK, QC], BF16, tag="exp", name=f"exp_{rep}_{h}_{qc}_{half}"
                    )
                    exp_halves.append(exp_h)
                    for kt in range(half * HK, (half + 1) * HK):
                        ps_st = psS.tile([P, QC], F32, tag="ps_s")
                        lhs = hat[hr:hr + 64, 3 + ht, kt * P:(kt + 1) * P]
                        rhs = hat[hr:hr + 64, ht, qc * QC:(qc + 1) * QC]
                        for nk in range(QC // MMW):
                            nc.tensor.matmul(
                                ps_st[:, nk * MMW:(nk + 1) * MMW],
                                lhs,
                                rhs[:, nk * MMW:(nk + 1) * MMW],
                                start=True,
                                stop=True,
                            )
                        if EXP_IMMB:
                            _act_imm(
                                nc, exp_h[:, kt - half * HK, 0:ABL_EXPW],
                                ps_st[:, 0:ABL_EXPW],
                                AF.Exp if ABL_EXPFN else AF.Copy,
                            )
                        else:
                            nc.scalar.activation(
                                exp_h[:, kt - half * HK, 0:ABL_EXPW],
                                ps_st[:, 0:ABL_EXPW],
                                AF.Exp if ABL_EXPFN else AF.Copy,
                            )
                return exp_halves

            def emit_pv(h, qc, exp_halves):
                ht = h // 2
                hr = 64 * (h % 2)
                for qw in range(QC // PO_W):
                    ps_o = psO.tile([65, PO_W], F32, tag="ps_o",
                                    name=f"pso_{rep}_{h}_{qc}_{qw}")
                    for kt in range(NKT):
                        for nk in range(PO_W // MMW):
                            qo = qw * PO_W + nk * MMW
                            nc.tensor.matmul(
                                ps_o[:, nk * MMW:(nk + 1) * MMW],
                                v_view[:, kt, h, :],
                                exp_halves[kt // HK][:, kt % HK, qo:qo + MMW],
                                start=(kt == 0),
                                stop=(kt == NKT - 1),
                            )
                    base = qc * QC + qw * PO_W
                    if ABL_EPI:
                        rc = pD.tile([1, PO_W], F32, tag="recip")
                        nc.vector.reciprocal(rc[:], ps_o[64:65, :])
                        rcb = pD.tile([64, PO_W], F32, tag="recipb")
                        nc.gpsimd.partition_broadcast(rcb[:], rc[:])
                        nc.vector.tensor_tensor(
                            out_t[hr:hr + 64, ht, base:base + PO_W],
                            ps_o[0:64, :],
                            rcb[:],
                            OP.mult,
                        )
                    else:
                        nc.vector.tensor_copy(
                            out_t[hr:hr + 64, ht, base:base + PO_W],
                            ps_o[0:64, :],
                        )

            # software pipeline: next chunk's S/exp is emitted before this
            # chunk's PV so the PE feeds ACT continuously.
            pending = None
            for h in range(ABL_HEADS if ABL_ATTN else 0):
                for qc in range(NQC):
                    eh = emit_s_exp(h, qc)
                    if pending is not None:
                        emit_pv(*pending)
                    pending = (h, qc, eh)
            if pending is not None:
                emit_pv(*pending)

        # ================ phase E: output projection ================
        with ExitStack() as sE:
            pE = sE.enter_context(tc.tile_pool(name=f"phE{rep}", bufs=2))
            wpp = sE.enter_context(tc.tile_pool(name=f"wpp{rep}", bufs=1))
            psE = sE.enter_context(tc.tile_pool(name=f"psE{rep}", bufs=2, space="PSUM"))
            wp_r = wpp.tile([P, KT3, C], F32R)
            nc.sync.dma_start(wp_r[:], wp_d.rearrange("(t p) f -> p t f", p=P))
            for mt in range(C // P if ABL_PROJ else 0):
                y_sb = pE.tile([P, N], F32, tag="y")
                for nk in range(N // 512):
                    ps_y = psE.tile([P, 512], F32, tag="ps_y")
                    for kt in range(KT3):
                        nc.tensor.matmul(
                            ps_y[:],
                            wp_r[:, kt, mt * P:(mt + 1) * P],
                            out_t[:, kt, nk * 512:(nk + 1) * 512],
                            start=(kt == 0),
                            stop=(kt == KT3 - 1),
                        )
                    # ACT is idle after attention; keep DVE free for the tail
                    _act_imm(nc, y_sb[:, nk * 512:(nk + 1) * 512], ps_y[:],
                             AF.Copy)
                nc.sync.dma_start(y_d[mt * P:(mt + 1) * P, :], y_sb[:])


def _emit_iteration_general(nc, tc, rep, cb,
                            x_d, wqk_d, wv_d, wp_d, y_d, ident, bd6, bc6, gb):
    """Reference-quality fallback: per-channel gamma and nonzero beta."""
    with ExitStack() as top:
        vp = top.enter_context(tc.tile_pool(name=f"vpool{rep}", bufs=1))
        v_sb = vp.tile([P, NKT, HL * 65], BF16)
        v_view = v_sb[:].rearrange("p t (h e) -> p t h e", h=HL)
        nc.gpsimd.memset(v_view[:, :, :, 64:65], 1.0)

        qkp = top.enter_context(tc.tile_pool(name=f"qkraw{rep}", bufs=1))
        qk_fts = [
            qkp.tile([P, N], F32R, name=f"qk_ft{ft}_{rep}") for ft in range(FT_QK)
        ]
        qk_raw = _FtView(qk_fts)
        sAC = top.enter_context(ExitStack())
        smp = sAC.enter_context(tc.tile_pool(name=f"smalls{rep}", bufs=1))

        # phase A
        with ExitStack() as sA:
            pA = sA.enter_context(tc.tile_pool(name=f"phA{rep}", bufs=1))
            pAx = sA.enter_context(tc.tile_pool(name=f"phAx{rep}", bufs=2))
            psA = sA.enter_context(tc.tile_pool(name=f"psA{rep}", bufs=2, space="PSUM"))

            wqk_r = pA.tile([P, CT, 2 * CL], F32R)
            nc.sync.dma_start(wqk_r[:], wqk_d.rearrange("(t p) f -> p t f", p=P))
            wv_r = pA.tile([P, CT, CL], F32R)
            nc.sync.dma_start(wv_r[:], wv_d.rearrange("(t p) f -> p t f", p=P))

            x_t = pA.tile([P, CT, N], F32R)
            for ct in range(CT):
                xs = pAx.tile([P, NKT, P], F32R, tag="xslice")
                nc.sync.dma_start(
                    xs[:], x_d.rearrange("(t p) c -> p t c", p=P)[:, :, ct * P:(ct + 1) * P]
                )
                for tg in range(NKT // 4):
                    pst = psA.tile([P, 4 * P], F32R, tag="ps_tr")
                    for i in range(4):
                        nc.tensor.transpose(
                            pst[:, i * P:(i + 1) * P], xs[:, tg * 4 + i, :], ident[:]
                        )
                    nc.vector.tensor_copy(
                        x_t[:, ct, tg * 512:(tg + 1) * 512], pst[:]
                    )

            for ft in range(FT_QK):
                for nk in range(N // 512):
                    ps = psA.tile([P, 512], F32, tag="ps_qkv")
                    for kt in range(CT):
                        nc.tensor.matmul(
                            ps[:],
                            wqk_r[:, kt, ft * P:(ft + 1) * P],
                            x_t[:, kt, nk * 512:(nk + 1) * 512],
                            start=(kt == 0),
                            stop=(kt == CT - 1),
                        )
                    nc.vector.tensor_copy(qk_raw[:, ft, nk * 512:(nk + 1) * 512], ps[:])

            for tt in range(NKT):
                psv = psA.tile([P, CL], F32, tag="ps_v")
                for kt in range(CT):
                    nc.tensor.matmul(
                        psv[:],
                        x_t[:, kt, tt * P:(tt + 1) * P],
                        wv_r[:, kt, :],
                        start=(kt == 0),
                        stop=(kt == CT - 1),
                    )
                nc.vector.tensor_copy(
                    v_view[:, tt, :, 0:64],
                    psv[:].rearrange("p (h d) -> p h d", h=HL),
                )

        # phase B
        with ExitStack() as sB:
            pB = sB.enter_context(tc.tile_pool(name=f"phB{rep}", bufs=2))
            psB = sB.enter_context(tc.tile_pool(name=f"psB{rep}", bufs=2, space="PSUM"))

            sm_mu = [smp.tile([HL, N], F32R, tag=f"mu{s}", name=f"sm_mu{s}_{rep}") for s in range(2)]
            sm_rst = [smp.tile([HL, N], F32R, tag=f"rst{s}", name=f"sm_rst{s}_{rep}") for s in range(2)]

            for s in range(2):
                for nh in range(N // 1024):
                    psm = psB.tile([HL, 1024], F32, tag="ps_stat")
                    for half in range(2):
                        nk = nh * 2 + half
                        for kt in range(KT3):
                            nc.tensor.matmul(
                                psm[:, half * 512:(half + 1) * 512],
                                bd6[:, kt, :],
                                qk_raw[:, 3 * s + kt, nk * 512:(nk + 1) * 512],
                                start=(kt == 0),
                                stop=(kt == KT3 - 1),
                            )
                    nc.vector.tensor_scalar_mul(
                        sm_mu[s][:, nh * 1024:(nh + 1) * 1024], psm[:], 1.0 / D
                    )
                for nh in range(N // 1024):
                    psm = psB.tile([HL, 1024], F32, tag="ps_stat")
                    for half in range(2):
                        nk = nh * 2 + half
                        for kt in range(KT3):
                            sq = pB.tile([P, 512], F32R, tag="sq")
                            nc.scalar.square(
                                sq[:], qk_raw[:, 3 * s + kt, nk * 512:(nk + 1) * 512]
                            )
                            nc.tensor.matmul(
                                psm[:, half * 512:(half + 1) * 512],
                                bd6[:, kt, :],
                                sq[:],
                                start=(kt == 0),
                                stop=(kt == KT3 - 1),
                            )
                    nc.vector.tensor_scalar_mul(
                        sm_rst[s][:, nh * 1024:(nh + 1) * 1024], psm[:], 1.0 / D
                    )
                tmp = smp.tile([HL, N], F32, tag=f"tmp{s}", name=f"tmp{s}_{rep}")
                nc.vector.tensor_tensor(tmp[:], sm_mu[s][:], sm_mu[s][:], OP.mult)
                nc.vector.scalar_tensor_tensor(
                    tmp[:], sm_rst[s][:], LN_EPS, tmp[:],
                    op0=OP.add, op1=OP.subtract,
                )
                nc.scalar.activation(tmp[:], tmp[:], AF.Sqrt)
                nc.vector.reciprocal(sm_rst[s][:], tmp[:])
                if s == 0:
                    nc.vector.tensor_scalar_mul(sm_rst[0][:], sm_rst[0][:], SCALE)

        # phase C: full LN apply on both sides
        hat = qk_raw
        with ExitStack() as sC:
            pC = sC.enter_context(tc.tile_pool(name=f"phC{rep}", bufs=2))
            psC = sC.enter_context(tc.tile_pool(name=f"psC{rep}", bufs=2, space="PSUM"))
            for ft in [0, 3, 1, 4, 2, 5]:
                s = ft // 3
                blk = ft % 3
                for nh in range(N // 1024):
                    sl = slice(nh * 1024, (nh + 1) * 1024)
                    bmu = psC.tile([P, 1024], F32, tag="bmu")
                    brs = psC.tile([P, 1024], F32, tag="brs")
                    for half in range(2):
                        hs = slice(half * 512, (half + 1) * 512)
                        gs = slice(nh * 1024 + half * 512,
                                   nh * 1024 + (half + 1) * 512)
                        nc.tensor.matmul(
                            bmu[:, hs], bc6[:, blk * P:(blk + 1) * P],
                            sm_mu[s][:, gs], start=True, stop=True,
                        )
                        nc.tensor.matmul(
                            brs[:, hs], bc6[:, blk * P:(blk + 1) * P],
                            sm_rst[s][:, gs], start=True, stop=True,
                        )
                    tdiff = pC.tile([P, 1024], F32, tag="tdiff")
                    nc.vector.tensor_tensor(
                        tdiff[:], qk_raw[:, ft, sl], bmu[:], OP.subtract
                    )
                    nc.vector.scalar_tensor_tensor(
                        hat[:, ft, sl],
                        tdiff[:],
                        gb[:, blk, 2 * s:2 * s + 1],
                        brs[:],
                        op0=OP.mult,
                        op1=OP.mult,
                    )
                    nc.vector.tensor_scalar_add(
                        hat[:, ft, sl], hat[:, ft, sl],
                        gb[:, blk, 2 * s + 1:2 * s + 2],
                    )

        sAC.close()

        # phase D
        outp = top.enter_context(tc.tile_pool(name=f"outT{rep}", bufs=1))
        out_fts = [
            outp.tile([P, N], F32R, name=f"out_ft{t}_{rep}") for t in range(KT3)
        ]
        out_t = _FtView(out_fts)
        with ExitStack() as sD:
            expp = sD.enter_context(tc.tile_pool(name=f"expp{rep}", bufs=2 * HPARTS))
            pD = sD.enter_context(tc.tile_pool(name=f"phD{rep}", bufs=2))
            psS = sD.enter_context(tc.tile_pool(name=f"psS{rep}", bufs=PSS_BUFS, space="PSUM"))
            psO = sD.enter_context(tc.tile_pool(name=f"psO{rep}", bufs=PSO_BUFS, space="PSUM"))
            HK = NKT // HPARTS

            def emit_s_exp(h, qc):
                ht = h // 2
                hr = 64 * (h % 2)
                exp_halves = []
                for half in range(HPARTS):
                    exp_h = expp.tile(
                        [P, HK, QC], BF16, tag="exp", name=f"exp_{rep}_{h}_{qc}_{half}"
                    )
                    exp_halves.append(exp_h)
                    for kt in range(half * HK, (half + 1) * HK):
                        ps_st = psS.tile([P, QC], F32, tag="ps_s")
                        lhs = hat[hr:hr + 64, 3 + ht, kt * P:(kt + 1) * P]
                        rhs = hat[hr:hr + 64, ht, qc * QC:(qc + 1) * QC]
                        for nk in range(QC // 512):
                            nc.tensor.matmul(
                                ps_st[:, nk * 512:(nk + 1) * 512],
                                lhs,
                                rhs[:, nk * 512:(nk + 1) * 512],
                                start=True,
                                stop=True,
                            )
                        nc.scalar.activation(
                            exp_h[:, kt - half * HK, :], ps_st[:], AF.Exp
                        )
                return exp_halves

            def emit_pv(h, qc, exp_halves):
                ht = h // 2
                hr = 64 * (h % 2)
                ps_o = psO.tile([65, QC], F32, tag="ps_o")
                for kt in range(NKT):
                    for nk in range(QC // 512):
                        nc.tensor.matmul(
                            ps_o[:, nk * 512:(nk + 1) * 512],
                            v_view[:, kt, h, :],
                            exp_halves[kt // HK][:, kt % HK,
                                                 nk * 512:(nk + 1) * 512],
                            start=(kt == 0),
                            stop=(kt == NKT - 1),
                        )
                rc = pD.tile([1, QC], F32, tag="recip")
                nc.vector.reciprocal(rc[:], ps_o[64:65, :])
                rcb = pD.tile([64, QC], F32, tag="recipb")
                nc.gpsimd.partition_broadcast(rcb[:], rc[:])
                nc.vector.tensor_tensor(
                    out_t[hr:hr + 64, ht, qc * QC:(qc + 1) * QC],
                    ps_o[0:64, :],
                    rcb[:],
                    OP.mult,
                )

            pending = None
            for h in range(HL):
                for qc in range(NQC):
                    eh = emit_s_exp(h, qc)
                    if pending is not None:
                        emit_pv(*pending)
                    pending = (h, qc, eh)
            if pending is not None:
                emit_pv(*pending)

        # phase E
        with ExitStack() as sE:
            pE = sE.enter_context(tc.tile_pool(name=f"phE{rep}", bufs=2))
            wpp = sE.enter_context(tc.tile_pool(name=f"wpp{rep}", bufs=1))
            psE = sE.enter_context(tc.tile_pool(name=f"psE{rep}", bufs=2, space="PSUM"))
            wp_r = wpp.tile([P, KT3, C], F32R)
            nc.sync.dma_start(wp_r[:], wp_d.rearrange("(t p) f -> p t f", p=P))
            for mt in range(C // P):
                y_sb = pE.tile([P, N], F32, tag="y")
                for nk in range(N // 512):
                    ps_y = psE.tile([P, 512], F32, tag="ps_y")
                    for kt in range(KT3):
                        nc.tensor.matmul(
                            ps_y[:],
                            wp_r[:, kt, mt * P:(mt + 1) * P],
                            out_t[:, kt, nk * 512:(nk + 1) * 512],
                            start=(kt == 0),
                            stop=(kt == KT3 - 1),
                        )
                    nc.vector.tensor_copy(y_sb[:, nk * 512:(nk + 1) * 512], ps_y[:])
                nc.sync.dma_start(y_d[mt * P:(mt + 1) * P, :], y_sb[:])


def _host_prep(x, w_qkv, q_gamma, q_beta, k_gamma, k_beta, w_proj):
    """Per-core input maps."""
    ident = np.eye(P, dtype=np.float32)
    bd6 = np.zeros((CL, HL), dtype=np.float32)
    for h in range(HL):
        bd6[h * D:(h + 1) * D, h] = 1.0
    bc6 = np.ascontiguousarray(bd6.T)
    in_maps = []
    for c in range(8):
        b = c // 2
        half = c % 2
        heads = range(HL * half, HL * half + HL)
        wq = np.concatenate([w_qkv[:, h * D:(h + 1) * D] for h in heads], axis=1)
        wk = np.concatenate(
            [w_qkv[:, C + h * D:C + (h + 1) * D] for h in heads], axis=1
        )
        wv = np.concatenate(
            [w_qkv[:, 2 * C + h * D:2 * C + (h + 1) * D] for h in heads], axis=1
        )
        wqk = np.ascontiguousarray(np.concatenate([wq, wk], axis=1))
        wp = np.ascontiguousarray(w_proj[CL * half:CL * half + CL, :])
        gb = np.stack(
            [
                np.tile(q_gamma, HL),
                np.tile(q_beta, HL) * SCALE,
                np.tile(k_gamma, HL),
                np.tile(k_beta, HL),
            ],
            axis=1,
        ).astype(np.float32)
        in_maps.append(
            {
                "x": np.ascontiguousarray(x[b]),
                "wqk": wqk,
                "wv": np.ascontiguousarray(wv),
                "wp": wp,
                "ident": ident,
                "bd6": bd6,
                "bc6": bc6,
                "gb": gb,
            }
        )
    return in_maps


def _pick_mode(q_gamma, q_beta, k_gamma, k_beta):
    """Fast path needs beta=0 and constant positive gammas (the centering
    cancellation requires per-channel-constant gamma)."""
    if not (np.all(q_beta == 0.0) and np.all(k_beta == 0.0)):
        return False
    gq, gk = float(q_gamma[0]), float(k_gamma[0])
    if not (np.all(q_gamma == gq) and np.all(k_gamma == gk)):
        return False
    if gq <= 0.0 or gk <= 0.0:
        return False
    return {"cq": math.log(SCALE * gq), "ck": math.log(gk)}


def kernel(x, w_qkv, q_gamma, q_beta, k_gamma, k_beta, w_proj, b_proj):
    x = np.asarray(x, dtype=np.float32)
    w_qkv = np.asarray(w_qkv, dtype=np.float32)
    q_gamma = np.asarray(q_gamma, dtype=np.float32)
    q_beta = np.asarray(q_beta, dtype=np.float32)
    k_gamma = np.asarray(k_gamma, dtype=np.float32)
    k_beta = np.asarray(k_beta, dtype=np.float32)
    w_proj = np.asarray(w_proj, dtype=np.float32)
    b_proj = np.asarray(b_proj, dtype=np.float32)

    mode = _pick_mode(q_gamma, q_beta, k_gamma, k_beta)
    nc = _build(mode)
    in_maps = _host_prep(x, w_qkv, q_gamma, q_beta, k_gamma, k_beta, w_proj)
    res = run_bass_kernel_spmd(nc, in_maps, core_ids=list(range(8)))

    y = np.empty((B, N, C), dtype=np.float32)
    for b in range(B):
        yt = res.results[2 * b]["y"] + res.results[2 * b + 1]["y"]
        y[b] = yt.T + b_proj[None, :]
    return y


if __name__ == "__main__":
    rng = np.random.default_rng(0)
    out = kernel(
        rng.standard_normal((B, N, C), dtype=np.float32),
        (rng.standard_normal((C, 3 * C)) * C ** -0.5).astype(np.float32),
        np.ones(D, np.float32),
        np.zeros(D, np.float32),
        np.ones(D, np.float32),
        np.zeros(D, np.float32),
        (rng.standard_normal((C, C)) * C ** -0.5).astype(np.float32),
        np.zeros(C, np.float32),
    )
    print("ok", out.shape, float(np.abs(out).mean()))
